# revision 11
# baseline (speedup 1.0000x reference)
"""Bandsplit module kernel for Trainium2 (8 NeuronCores, SPMD data-parallel).

Math (reference):
    x: (B=16, C=2, F=2048, T=1024) f32
    xb = x.reshape(B, C, 64, 32, T); xm = xb.mean(axis=3)        # (B, C, 64, T)
    out = einsum('bcnt,nce->bnte', xm, W) + b[None, :, None, :]   # (B, 64, T, 128)

Strategy:
  - Data-parallel over batch: 16 / 8 cores = 2 batches per core. Per-band
    weights are tiny and replicated.
  - The band-mean and the per-band linear projection fuse into PE matmuls:
    for each (batch, band, t-chunk of 128), contract K = (f, c) = 64 rows of
    x against a host-precomputed [64, 128] block W[n, c, e] / 32.  Output
    [t, e] lands in PSUM already in the output layout.
  - fp32 matmul on TRN2 runs at 4 cycles/row; instead x and W/32 are split
    host-side into bf16 hi + lo parts and each t-chunk does 2 bf16 K=128
    matmuls accumulating in fp32 PSUM: [xh;xl]@[wh;wh] + [xh;xl]@[wl;0]
    = xh*wh + xl*wh + xh*wl (the dropped xl*wl term is ~2^-16 relative).
    ~fp32-grade results at bf16 speed.  K=128 matters beyond density: the
    PE's HAM clock-gate never leaves the cold 1.2 GHz state for K=64
    matmul streams (measured), but warms to 2.4 GHz at K>=96.
  - x ships as a host-packed [128, T] bf16 tile per (batch, band): hi in
    partitions 0-63 (k = f*2+c), lo in partitions 64-127. Same bytes as
    fp32, one DMA per tile with 2KB-contiguous rows spread across all 16
    SDMA engines (outer-dim split rule).
  - 4 t-chunk matmul groups accumulate into one [128, 512] PSUM bank; a
    single vector-engine tensor_add per bank fuses the bias (free-dim
    step-0 broadcast of the replicated bias tile) with the PSUM->SBUF move.
  - Input DMAs issue on the sync (SP) HWDGE ring, output DMAs on the
    scalar (ACT) ring, so neither sequencer's ~0.7us/DMA issue cost stacks.
"""

import sys

import numpy as np

if "/opt/trn_rl_repo" not in sys.path:
    sys.path.insert(0, "/opt/trn_rl_repo")

import ml_dtypes

BF16 = ml_dtypes.bfloat16

B, C, F, T = 16, 2, 2048, 1024
N_BANDS, BAND, EMB = 64, 32, 128
K = C * BAND  # contraction rows from x per band
N_CORES = 8
B_LOC = B // N_CORES
TCH = T // 128  # t-chunks of 128 per band
QUAD = 1024 // EMB  # t-chunks per PSUM tile (2 banks)

_CACHE: dict = {}


def _build_nc():
    import concourse.mybir as mybir
    from concourse import bacc
    from concourse.bass import ds, ts
    from concourse.tile import TileContext

    f32 = mybir.dt.float32
    bf16 = mybir.dt.bfloat16
    nc = bacc.Bacc("TRN2", target_bir_lowering=False, debug=False, num_devices=N_CORES)

    # x packed host-side: [b, n, p, t]; p<64 -> bf16 hi (k = f*2+c), p>=64 -> lo
    xp = nc.dram_tensor("xp", [B_LOC, N_BANDS // 2, 2 * K, 2 * T], bf16, kind="ExternalInput").ap()
    w1 = nc.dram_tensor("w1", [2 * K, N_BANDS * EMB], bf16, kind="ExternalInput").ap()
    w2 = nc.dram_tensor("w2", [2 * K, N_BANDS * EMB], bf16, kind="ExternalInput").ap()
    bb = nc.dram_tensor("bb", [128, N_BANDS * EMB], f32, kind="ExternalInput").ap()
    out = nc.dram_tensor("out", [B_LOC, N_BANDS, T, EMB], f32, kind="ExternalOutput").ap()

    # out per (batch, band) as [p, tc, e] with t = tc*128 + p
    ov = out.rearrange("b n (tc p) e -> b n p tc e", p=128)

    with TileContext(nc) as tc:
        with (
            tc.tile_pool(name="wpool", bufs=1) as wpool,
            tc.tile_pool(name="xpool", bufs=4) as xpool,
            tc.tile_pool(name="opool", bufs=3) as opool,
            tc.tile_pool(name="ppool", bufs=4, space="PSUM") as ppool,
        ):
            w1t = wpool.tile([2 * K, N_BANDS * EMB], bf16)
            nc.sync.dma_start(w1t[:], w1[:])
            w2t = wpool.tile([2 * K, N_BANDS * EMB], bf16)
            nc.sync.dma_start(w2t[:], w2[:])
            bt = wpool.tile([128, N_BANDS * EMB], f32)
            nc.sync.dma_start(bt[:], bb[:])

            for b in range(B_LOC):
                for np_ in range(N_BANDS // 2):
                    xt = xpool.tile([2 * K, 2 * T], bf16)
                    nc.sync.dma_start(xt[:], xp[b, np_])

                    for g in range(2):
                        n = 2 * np_ + g
                        bias = (
                            bt[:, ts(n, EMB)]
                            .unsqueeze(1)
                            .broadcast_to([128, QUAD, EMB])
                        )
                        osb = opool.tile([128, T], f32)
                        for q in range(TCH // QUAD):
                            ps = ppool.tile([128, QUAD * EMB], f32)
                            for j in range(QUAD):
                                ti = q * QUAD + j
                                x_c = xt[:, ds(g * T + ti * 128, 128)]
                                nc.tensor.matmul(
                                    ps[:, ts(j, EMB)], x_c, w1t[:, ts(n, EMB)],
                                    start=True, stop=False,
                                )
                                nc.tensor.matmul(
                                    ps[:, ts(j, EMB)], x_c, w2t[:, ts(n, EMB)],
                                    start=False, stop=True,
                                )
                            nc.vector.tensor_add(
                                osb[:, ts(q, QUAD * EMB)], ps[:], bias
                            )

                        nc.scalar.dma_start(ov[b, n], osb[:])

    nc.compile()
    return nc


def _get_nc():
    if "nc" not in _CACHE:
        _CACHE["nc"] = _build_nc()
    return _CACHE["nc"]


def _host_prep(x: np.ndarray, W: np.ndarray, b: np.ndarray):
    x = np.asarray(x, np.float32)
    # bf16 hi/lo split of x, rearranged to [b, n, (f c | f c), t]
    xh = x.astype(BF16)
    xl = (x - xh.astype(np.float32)).astype(BF16)

    def pack(a):
        # (B, C, F, T) -> (B, n, f, c, t) -> (B, n, K, T)
        return (
            a.reshape(B, C, N_BANDS, BAND, T)
            .transpose(0, 2, 3, 1, 4)
            .reshape(B, N_BANDS, K, T)
        )

    xp = np.concatenate([pack(xh), pack(xl)], axis=2)  # (B, n, 2K, T) bf16
    # pair adjacent bands along the row axis: (B, n/2, 2K, 2T) -> 4KB rows
    xp = (
        xp.reshape(B, N_BANDS // 2, 2, 2 * K, T)
        .transpose(0, 1, 3, 2, 4)
        .reshape(B, N_BANDS // 2, 2 * K, 2 * T)
    )

    # w[k = f*2+c, n*EMB+e] = W[n, c, e] / BAND, split hi/lo
    wc = (np.asarray(W, np.float32).transpose(1, 0, 2) / BAND).astype(np.float32)
    wkf = np.broadcast_to(wc[None], (BAND, C, N_BANDS, EMB)).reshape(K, N_BANDS * EMB)
    wh = wkf.astype(BF16)
    wl = (wkf - wh.astype(np.float32)).astype(BF16)
    w1 = np.concatenate([wh, wh], axis=0)                  # [2K, n*e]
    w2 = np.concatenate([wl, np.zeros_like(wl)], axis=0)   # [2K, n*e]

    bb = np.broadcast_to(
        np.asarray(b, np.float32).reshape(1, N_BANDS * EMB), (128, N_BANDS * EMB)
    )
    return (
        np.ascontiguousarray(xp),
        np.ascontiguousarray(w1),
        np.ascontiguousarray(w2),
        np.ascontiguousarray(bb),
    )


def kernel(x: np.ndarray, W: np.ndarray, b: np.ndarray, _trace: bool = False):
    from concourse.bass_utils import run_bass_kernel_spmd

    nc = _get_nc()
    xp, w1, w2, bb = _host_prep(x, W, b)

    in_maps = [
        {"xp": xp[i * B_LOC : (i + 1) * B_LOC], "w1": w1, "w2": w2, "bb": bb}
        for i in range(N_CORES)
    ]
    res = run_bass_kernel_spmd(nc, in_maps, core_ids=list(range(N_CORES)), trace=_trace)
    out = np.concatenate([r["out"] for r in res.results], axis=0)
    if _trace:
        _CACHE["last_exec_time_ns"] = res.exec_time_ns
    return out


# revision 12
# speedup vs baseline: 1.0554x; 1.0554x over previous
"""Bandsplit module kernel for Trainium2 (8 NeuronCores, SPMD data-parallel).

Math (reference):
    x: (B=16, C=2, F=2048, T=1024) f32
    xb = x.reshape(B, C, 64, 32, T); xm = xb.mean(axis=3)        # (B, C, 64, T)
    out = einsum('bcnt,nce->bnte', xm, W) + b[None, :, None, :]   # (B, 64, T, 128)

Strategy:
  - Data-parallel over batch: 16 / 8 cores = 2 batches per core. Per-band
    weights are tiny and replicated.
  - The band-mean and the per-band linear projection fuse into PE matmuls:
    for each (batch, band, t-chunk of 128), contract K = (f, c) = 64 rows of
    x against a host-precomputed [64, 128] block W[n, c, e] / 32.  Output
    [t, e] lands in PSUM already in the output layout.
  - fp32 matmul on TRN2 runs at 4 cycles/row; instead x and W/32 are split
    host-side into bf16 hi + lo parts and each t-chunk does 2 bf16 K=128
    matmuls accumulating in fp32 PSUM: [xh;xl]@[wh;wh] + [xh;xl]@[wl;0]
    = xh*wh + xl*wh + xh*wl (the dropped xl*wl term is ~2^-16 relative).
    ~fp32-grade results at bf16 speed.  K=128 matters beyond density: the
    PE's HAM clock-gate never leaves the cold 1.2 GHz state for K=64
    matmul streams (measured), but warms to 2.4 GHz at K>=96.
  - x ships as a host-packed [128, T] bf16 tile per (batch, band): hi in
    partitions 0-63 (k = f*2+c), lo in partitions 64-127. Same bytes as
    fp32, one DMA per tile with 2KB-contiguous rows spread across all 16
    SDMA engines (outer-dim split rule).
  - 4 t-chunk matmul groups accumulate into one [128, 512] PSUM bank; a
    single vector-engine tensor_add per bank fuses the bias (free-dim
    step-0 broadcast of the replicated bias tile) with the PSUM->SBUF move.
  - Input DMAs issue on the sync (SP) HWDGE ring, output DMAs on the
    scalar (ACT) ring, so neither sequencer's ~0.7us/DMA issue cost stacks.
"""

import sys

import numpy as np

if "/opt/trn_rl_repo" not in sys.path:
    sys.path.insert(0, "/opt/trn_rl_repo")

import ml_dtypes

BF16 = ml_dtypes.bfloat16

B, C, F, T = 16, 2, 2048, 1024
N_BANDS, BAND, EMB = 64, 32, 128
K = C * BAND  # contraction rows from x per band
N_CORES = 8
B_LOC = B // N_CORES
TCH = T // 128  # t-chunks of 128 per band
QUAD = 1024 // EMB  # t-chunks per PSUM tile (2 banks)

_CACHE: dict = {}


def _build_nc():
    import concourse.mybir as mybir
    from concourse import bacc
    from concourse.bass import ds, ts
    from concourse.tile import TileContext

    f32 = mybir.dt.float32
    bf16 = mybir.dt.bfloat16
    nc = bacc.Bacc("TRN2", target_bir_lowering=False, debug=False, num_devices=N_CORES)

    # x packed host-side: [b, n, p, t]; p<64 -> bf16 hi (k = f*2+c), p>=64 -> lo
    xp = nc.dram_tensor("xp", [B_LOC, N_BANDS // 2, 2 * K, 2 * T], bf16, kind="ExternalInput").ap()
    w1 = nc.dram_tensor("w1", [2 * K, N_BANDS * EMB], bf16, kind="ExternalInput").ap()
    w2 = nc.dram_tensor("w2", [2 * K, N_BANDS * EMB], bf16, kind="ExternalInput").ap()
    bb = nc.dram_tensor("bb", [128, N_BANDS * EMB], f32, kind="ExternalInput").ap()
    out = nc.dram_tensor("out", [B_LOC, N_BANDS, T, EMB], f32, kind="ExternalOutput").ap()

    # out per (batch, band) as [p, j, e] with t = 8*p + j: each SBUF
    # partition holds 8 consecutive t rows = a 4KB-contiguous DRAM run,
    # so output DMA descriptors are 4KB instead of 512B (x tile t-columns
    # are host-permuted so matmul chunk j covers t === j (mod 8))
    ov = out.rearrange("b n (p j) e -> b n p j e", j=TCH)

    with TileContext(nc) as tc:
        with (
            tc.tile_pool(name="wpool", bufs=1) as wpool,
            tc.tile_pool(name="xpool", bufs=4) as xpool,
            tc.tile_pool(name="opool", bufs=3) as opool,
            tc.tile_pool(name="ppool", bufs=4, space="PSUM") as ppool,
        ):
            w1t = wpool.tile([2 * K, N_BANDS * EMB], bf16)
            nc.sync.dma_start(w1t[:], w1[:])
            w2t = wpool.tile([2 * K, N_BANDS * EMB], bf16)
            nc.sync.dma_start(w2t[:], w2[:])
            bt = wpool.tile([128, N_BANDS * EMB], f32)
            nc.sync.dma_start(bt[:], bb[:])

            for b in range(B_LOC):
                for np_ in range(N_BANDS // 2):
                    xt = xpool.tile([2 * K, 2 * T], bf16)
                    nc.sync.dma_start(xt[:], xp[b, np_])

                    for g in range(2):
                        n = 2 * np_ + g
                        bias = (
                            bt[:, ts(n, EMB)]
                            .unsqueeze(1)
                            .broadcast_to([128, QUAD, EMB])
                        )
                        osb = opool.tile([128, T], f32)
                        for q in range(TCH // QUAD):
                            ps = ppool.tile([128, QUAD * EMB], f32)
                            for j in range(QUAD):
                                ti = q * QUAD + j
                                x_c = xt[:, ds(g * T + ti * 128, 128)]
                                nc.tensor.matmul(
                                    ps[:, ts(j, EMB)], x_c, w1t[:, ts(n, EMB)],
                                    start=True, stop=False,
                                )
                                nc.tensor.matmul(
                                    ps[:, ts(j, EMB)], x_c, w2t[:, ts(n, EMB)],
                                    start=False, stop=True,
                                )
                            nc.vector.tensor_add(
                                osb[:, ts(q, QUAD * EMB)], ps[:], bias
                            )

                        nc.scalar.dma_start(ov[b, n], osb[:])

    nc.compile()
    return nc


def _get_nc():
    if "nc" not in _CACHE:
        _CACHE["nc"] = _build_nc()
    return _CACHE["nc"]


def _host_prep(x: np.ndarray, W: np.ndarray, b: np.ndarray):
    x = np.asarray(x, np.float32)
    # bf16 hi/lo split of x, rearranged to [b, n, (f c | f c), t]
    xh = x.astype(BF16)
    xl = (x - xh.astype(np.float32)).astype(BF16)

    def pack(a):
        # (B, C, F, T) -> (B, n, f, c, t) -> (B, n, K, T)
        return (
            a.reshape(B, C, N_BANDS, BAND, T)
            .transpose(0, 2, 3, 1, 4)
            .reshape(B, N_BANDS, K, T)
        )

    xp = np.concatenate([pack(xh), pack(xl)], axis=2)  # (B, n, 2K, T) bf16
    # pair adjacent bands along the row axis: (B, n/2, 2K, 2T) -> 4KB rows
    # permute t within each band so chunk j holds t === j (mod 8):
    # col (j, p) <- t = 8p + j
    xp = (
        xp.reshape(B, N_BANDS, 2 * K, T // TCH, TCH)
        .transpose(0, 1, 2, 4, 3)
        .reshape(B, N_BANDS, 2 * K, T)
    )
    # pair adjacent bands along the row axis: (B, n/2, 2K, 2T) -> 4KB rows
    xp = (
        xp.reshape(B, N_BANDS // 2, 2, 2 * K, T)
        .transpose(0, 1, 3, 2, 4)
        .reshape(B, N_BANDS // 2, 2 * K, 2 * T)
    )

    # w[k = f*2+c, n*EMB+e] = W[n, c, e] / BAND, split hi/lo
    wc = (np.asarray(W, np.float32).transpose(1, 0, 2) / BAND).astype(np.float32)
    wkf = np.broadcast_to(wc[None], (BAND, C, N_BANDS, EMB)).reshape(K, N_BANDS * EMB)
    wh = wkf.astype(BF16)
    wl = (wkf - wh.astype(np.float32)).astype(BF16)
    w1 = np.concatenate([wh, wh], axis=0)                  # [2K, n*e]
    w2 = np.concatenate([wl, np.zeros_like(wl)], axis=0)   # [2K, n*e]

    bb = np.broadcast_to(
        np.asarray(b, np.float32).reshape(1, N_BANDS * EMB), (128, N_BANDS * EMB)
    )
    return (
        np.ascontiguousarray(xp),
        np.ascontiguousarray(w1),
        np.ascontiguousarray(w2),
        np.ascontiguousarray(bb),
    )


def kernel(x: np.ndarray, W: np.ndarray, b: np.ndarray, _trace: bool = False):
    from concourse.bass_utils import run_bass_kernel_spmd

    nc = _get_nc()
    xp, w1, w2, bb = _host_prep(x, W, b)

    in_maps = [
        {"xp": xp[i * B_LOC : (i + 1) * B_LOC], "w1": w1, "w2": w2, "bb": bb}
        for i in range(N_CORES)
    ]
    res = run_bass_kernel_spmd(nc, in_maps, core_ids=list(range(N_CORES)), trace=_trace)
    out = np.concatenate([r["out"] for r in res.results], axis=0)
    if _trace:
        _CACHE["last_exec_time_ns"] = res.exec_time_ns
    return out


# revision 13
# speedup vs baseline: 1.1298x; 1.0705x over previous
"""Bandsplit module kernel for Trainium2 (8 NeuronCores, SPMD data-parallel).

Math (reference):
    x: (B=16, C=2, F=2048, T=1024) f32
    xb = x.reshape(B, C, 64, 32, T); xm = xb.mean(axis=3)        # (B, C, 64, T)
    out = einsum('bcnt,nce->bnte', xm, W) + b[None, :, None, :]   # (B, 64, T, 128)

Strategy:
  - Data-parallel over batch: 16 / 8 cores = 2 batches per core. Per-band
    weights are tiny and replicated.
  - The band-mean and the per-band linear projection fuse into PE matmuls:
    for each (batch, band, t-chunk of 128), contract K = (f, c) = 64 rows of
    x against a host-precomputed [64, 128] block W[n, c, e] / 32.  Output
    [t, e] lands in PSUM already in the output layout.
  - fp32 matmul on TRN2 runs at 4 cycles/row; instead x and W/32 are split
    host-side into bf16 hi + lo parts and each t-chunk does 2 bf16 K=128
    matmuls accumulating in fp32 PSUM: [xh;xl]@[wh;wh] + [xh;xl]@[wl;0]
    = xh*wh + xl*wh + xh*wl (the dropped xl*wl term is ~2^-16 relative).
    ~fp32-grade results at bf16 speed.  K=128 matters beyond density: the
    PE's HAM clock-gate never leaves the cold 1.2 GHz state for K=64
    matmul streams (measured), but warms to 2.4 GHz at K>=96.
  - x ships as a host-packed [128, T] bf16 tile per (batch, band): hi in
    partitions 0-63 (k = f*2+c), lo in partitions 64-127. Same bytes as
    fp32, one DMA per tile with 2KB-contiguous rows spread across all 16
    SDMA engines (outer-dim split rule).
  - 4 t-chunk matmul groups accumulate into one [128, 512] PSUM bank; a
    single vector-engine tensor_add per bank fuses the bias (free-dim
    step-0 broadcast of the replicated bias tile) with the PSUM->SBUF move.
  - Input DMAs issue on the sync (SP) HWDGE ring, output DMAs on the
    scalar (ACT) ring, so neither sequencer's ~0.7us/DMA issue cost stacks.
"""

import sys

import numpy as np

if "/opt/trn_rl_repo" not in sys.path:
    sys.path.insert(0, "/opt/trn_rl_repo")

import ml_dtypes

BF16 = ml_dtypes.bfloat16

B, C, F, T = 16, 2, 2048, 1024
N_BANDS, BAND, EMB = 64, 32, 128
K = C * BAND  # contraction rows from x per band
N_CORES = 8
B_LOC = B // N_CORES
TCH = T // 128  # t-chunks of 128 per band
QUAD = 1024 // EMB  # t-chunks per PSUM tile (2 banks)

_CACHE: dict = {}


def _build_nc():
    import concourse.mybir as mybir
    from concourse import bacc
    from concourse.bass import ds, ts
    from concourse.tile import TileContext

    f32 = mybir.dt.float32
    bf16 = mybir.dt.bfloat16
    nc = bacc.Bacc("TRN2", target_bir_lowering=False, debug=False, num_devices=N_CORES)

    # x packed host-side: [b, n, p, t]; p<64 -> bf16 hi (k = f*2+c), p>=64 -> lo
    xp = nc.dram_tensor("xp", [B_LOC, N_BANDS // 4, 2 * K, 4 * T], bf16, kind="ExternalInput").ap()
    w1 = nc.dram_tensor("w1", [2 * K, N_BANDS * EMB], bf16, kind="ExternalInput").ap()
    w2 = nc.dram_tensor("w2", [2 * K, N_BANDS * EMB], bf16, kind="ExternalInput").ap()
    bb = nc.dram_tensor("bb", [1, N_BANDS * EMB], f32, kind="ExternalInput").ap()
    out = nc.dram_tensor("out", [B_LOC, N_BANDS, T, EMB], f32, kind="ExternalOutput").ap()

    # out per (batch, band) as [p, j, e] with t = 8*p + j: each SBUF
    # partition holds 8 consecutive t rows = a 4KB-contiguous DRAM run,
    # so output DMA descriptors are 4KB instead of 512B (x tile t-columns
    # are host-permuted so matmul chunk j covers t === j (mod 8))
    ov = out.rearrange("b n (p j) e -> b n p j e", j=TCH)

    with TileContext(nc) as tc:
        with (
            tc.tile_pool(name="wpool", bufs=1) as wpool,
            tc.tile_pool(name="xpool", bufs=4) as xpool,
            tc.tile_pool(name="opool", bufs=3) as opool,
            tc.tile_pool(name="ppool", bufs=4, space="PSUM") as ppool,
        ):
            w1t = wpool.tile([2 * K, N_BANDS * EMB], bf16)
            nc.scalar.dma_start(w1t[:], w1[:])
            w2t = wpool.tile([2 * K, N_BANDS * EMB], bf16)
            nc.scalar.dma_start(w2t[:], w2[:])
            bsm = wpool.tile([1, N_BANDS * EMB], f32)
            nc.sync.dma_start(bsm[:], bb[:])
            bt = wpool.tile([128, N_BANDS * EMB], f32)
            nc.gpsimd.partition_broadcast(bt[:], bsm[:])

            for b in range(B_LOC):
                for np_ in range(N_BANDS // 4):
                    xt = xpool.tile([2 * K, 4 * T], bf16)
                    nc.sync.dma_start(xt[:], xp[b, np_])

                    for g in range(4):
                        n = 4 * np_ + g
                        bias = (
                            bt[:, ts(n, EMB)]
                            .unsqueeze(1)
                            .broadcast_to([128, QUAD, EMB])
                        )
                        osb = opool.tile([128, T], f32)
                        for q in range(TCH // QUAD):
                            ps = ppool.tile([128, QUAD * EMB], f32)
                            for j in range(QUAD):
                                ti = q * QUAD + j
                                x_c = xt[:, ds(g * T + ti * 128, 128)]
                                nc.tensor.matmul(
                                    ps[:, ts(j, EMB)], x_c, w1t[:, ts(n, EMB)],
                                    start=True, stop=False,
                                )
                                nc.tensor.matmul(
                                    ps[:, ts(j, EMB)], x_c, w2t[:, ts(n, EMB)],
                                    start=False, stop=True,
                                )
                            nc.vector.tensor_add(
                                osb[:, ts(q, QUAD * EMB)], ps[:], bias
                            )

                        nc.scalar.dma_start(ov[b, n], osb[:])

    nc.compile()
    return nc


def _get_nc():
    if "nc" not in _CACHE:
        _CACHE["nc"] = _build_nc()
    return _CACHE["nc"]


def _host_prep(x: np.ndarray, W: np.ndarray, b: np.ndarray):
    x = np.asarray(x, np.float32)
    # bf16 hi/lo split of x, rearranged to [b, n, (f c | f c), t]
    xh = x.astype(BF16)
    xl = (x - xh.astype(np.float32)).astype(BF16)

    def pack(a):
        # (B, C, F, T) -> (B, n, f, c, t) -> (B, n, K, T)
        return (
            a.reshape(B, C, N_BANDS, BAND, T)
            .transpose(0, 2, 3, 1, 4)
            .reshape(B, N_BANDS, K, T)
        )

    xp = np.concatenate([pack(xh), pack(xl)], axis=2)  # (B, n, 2K, T) bf16
    # pair adjacent bands along the row axis: (B, n/2, 2K, 2T) -> 4KB rows
    # permute t within each band so chunk j holds t === j (mod 8):
    # col (j, p) <- t = 8p + j
    xp = (
        xp.reshape(B, N_BANDS, 2 * K, T // TCH, TCH)
        .transpose(0, 1, 2, 4, 3)
        .reshape(B, N_BANDS, 2 * K, T)
    )
    # group 4 adjacent bands along the row axis: (B, n/4, 2K, 4T) -> 8KB rows
    xp = (
        xp.reshape(B, N_BANDS // 4, 4, 2 * K, T)
        .transpose(0, 1, 3, 2, 4)
        .reshape(B, N_BANDS // 4, 2 * K, 4 * T)
    )

    # w[k = f*2+c, n*EMB+e] = W[n, c, e] / BAND, split hi/lo
    wc = (np.asarray(W, np.float32).transpose(1, 0, 2) / BAND).astype(np.float32)
    wkf = np.broadcast_to(wc[None], (BAND, C, N_BANDS, EMB)).reshape(K, N_BANDS * EMB)
    wh = wkf.astype(BF16)
    wl = (wkf - wh.astype(np.float32)).astype(BF16)
    w1 = np.concatenate([wh, wh], axis=0)                  # [2K, n*e]
    w2 = np.concatenate([wl, np.zeros_like(wl)], axis=0)   # [2K, n*e]

    bb = np.asarray(b, np.float32).reshape(1, N_BANDS * EMB)
    return (
        np.ascontiguousarray(xp),
        np.ascontiguousarray(w1),
        np.ascontiguousarray(w2),
        np.ascontiguousarray(bb),
    )


def kernel(x: np.ndarray, W: np.ndarray, b: np.ndarray, _trace: bool = False):
    from concourse.bass_utils import run_bass_kernel_spmd

    nc = _get_nc()
    xp, w1, w2, bb = _host_prep(x, W, b)

    in_maps = [
        {"xp": xp[i * B_LOC : (i + 1) * B_LOC], "w1": w1, "w2": w2, "bb": bb}
        for i in range(N_CORES)
    ]
    res = run_bass_kernel_spmd(nc, in_maps, core_ids=list(range(N_CORES)), trace=_trace)
    out = np.concatenate([r["out"] for r in res.results], axis=0)
    if _trace:
        _CACHE["last_exec_time_ns"] = res.exec_time_ns
    return out
